# revision 9
# baseline (speedup 1.0000x reference)
"""Masked dot-product attention on 8 Trainium2 NeuronCores.

Problem: B=8, S=4096, D=64 fp32; per-batch key-length mask; softmax over keys.

Sharding: sequence-parallel over Q rows. Each core computes a 512-row Q slice
of all 8 batches. The key loop for batch b runs ceil(valid_len[b]/128) tiles
(same trip counts on every core -> one SPMD program, perfectly balanced
regardless of the valid_len distribution).

Per (batch, core) unit, with scores kept in transposed [k, q] layout:
  phase 1: psum_s[k=128, q=512] = K_tile.T @ Q  (bf16, fast weight load)
  exp:     one ScalarE activation per group of 3 k-tiles, exp(0.125 * s),
           PSUM -> SBUF bf16. No max-subtraction: scores ~ N(0,1) after the
           1/8 scale, exp is safe in fp32.
  phase 2: psum_o[128, q=512] += V_tile.T @ exp_tile in bf16. V is padded to
           128 weight columns (FWL) with col 64 = ones, so row 64 of psum_o
           accumulates the softmax denominator.
  tail:    r = 1/psum_o[64] (DVE), partition-broadcast r (GpSimd), multiply
           rows 0..63 (DVE), DMA out in [d, q] layout; host transposes back.

Masking costs nothing on-device: the host zeroes V rows (incl. the ones
column) at key positions >= valid_len, so masked keys contribute 0 to both
numerator and denominator; exp of their scores is finite garbage times zero.

Perf notes baked in below: per-batch coalesced DMAs (small DMAs are
descriptor-dominated and serialize on the issue queue); a scratch-matmul
warm-up burst so the PE HAM clock gate opens (1.2 -> 2.4 GHz) before real
work; smallest batch first so compute starts behind a short first DMA;
double-buffered 3-bank PSUM score groups keep PE and ACT both dense.
"""

import math
from contextlib import ExitStack

import numpy as np

B = 8
S = 4096
D = 64
N_CORES = 8
QB = S // N_CORES  # 512 q rows per core per batch
KT = 128  # k rows per tile
NKMAX = S // KT  # 32
GROUP = 3  # k-tiles per PSUM group / ACT instruction
SCALE = 1.0 / math.sqrt(D)

_PROGRAM_CACHE: dict = {}


def _build_program(k_tiles):
    import concourse.tile as tile
    from concourse import bacc, mybir

    f32 = mybir.dt.float32
    bf16 = mybir.dt.bfloat16
    nc = bacc.Bacc("TRN2", target_bir_lowering=False, debug=False,
                   enable_asserts=False, num_devices=N_CORES)

    qx = nc.dram_tensor("qx", [D, B * QB], bf16, kind="ExternalInput").ap()
    kx = nc.dram_tensor("kx", [B, D, S], bf16, kind="ExternalInput").ap()
    vx = nc.dram_tensor("vx", [B, KT, NKMAX, KT], bf16,
                        kind="ExternalInput").ap()
    out = nc.dram_tensor("out", [B, D, QB], f32, kind="ExternalOutput").ap()

    with tile.TileContext(nc) as tc:
        with ExitStack() as ctx:
            q_pool = ctx.enter_context(tc.tile_pool(name="q", bufs=1))
            k_pool = ctx.enter_context(tc.tile_pool(name="k", bufs=2))
            v_pool = ctx.enter_context(tc.tile_pool(name="v", bufs=2))
            e_pool = ctx.enter_context(tc.tile_pool(name="e", bufs=3))
            n_pool = ctx.enter_context(tc.tile_pool(name="n", bufs=2))
            ps_s_pool = ctx.enter_context(
                tc.tile_pool(name="ps_s", bufs=2, space="PSUM"))
            ps_o_pool = ctx.enter_context(
                tc.tile_pool(name="ps_o", bufs=2, space="PSUM"))

            q_all = q_pool.tile([D, B * QB], bf16)
            nc.sync.dma_start(q_all[:], qx[:])

            # HAM warm-up: dense scratch matmuls while the first DMAs land,
            # so the PE clock ungates (1.2 -> 2.4 GHz) before real work.
            wu_sb = q_pool.tile([D, QB], bf16, tag="warm", bufs=1)
            nc.gpsimd.memset(wu_sb[:], 0.0)
            ps_w = ps_o_pool.tile([KT, QB], f32, tag="ps_o")
            for _ in range(24):
                nc.tensor.matmul(ps_w[:], lhsT=wu_sb[:, :KT],
                                 rhs=wu_sb[:], start=True, stop=True)

            # largest batches first; the exposed tail batch is the smallest.
            for b in sorted(range(B), key=lambda x: -k_tiles[x]):
                nk = k_tiles[b]
                ngroups = (nk + GROUP - 1) // GROUP

                k_all = k_pool.tile([D, S], bf16)
                nc.sync.dma_start(k_all[:, :nk * KT], kx[b][:, :nk * KT])
                v_all = v_pool.tile([KT, NKMAX * KT], bf16)
                nc.sync.dma_start(
                    v_all[:, :nk * KT].rearrange("p (t c) -> p t c", c=KT),
                    vx[b][:, :nk, :])

                ps_o = ps_o_pool.tile([KT, QB], f32, tag="ps_o")
                q_b = q_all[:, b * QB:(b + 1) * QB]

                for g in range(ngroups):
                    tg = min(GROUP, nk - g * GROUP)
                    ps_s = ps_s_pool.tile([KT, GROUP * QB], f32)
                    e_sb = e_pool.tile([KT, GROUP * QB], bf16)
                    for tl in range(tg):
                        kt = g * GROUP + tl
                        nc.tensor.matmul(
                            ps_s[:, tl * QB:(tl + 1) * QB],
                            lhsT=k_all[:, kt * KT:(kt + 1) * KT],
                            rhs=q_b,
                            start=True, stop=True)
                    nc.scalar.activation(
                        e_sb[:, :tg * QB], ps_s[:, :tg * QB],
                        mybir.ActivationFunctionType.Exp, scale=SCALE)
                    for tl in range(tg):
                        kt = g * GROUP + tl
                        nc.tensor.matmul(
                            ps_o[:],
                            lhsT=v_all[:, kt * KT:(kt + 1) * KT],
                            rhs=e_sb[:, tl * QB:(tl + 1) * QB],
                            start=(kt == 0), stop=(kt == nk - 1),
                            skip_group_check=True)

                r_row = n_pool.tile([1, QB], f32, tag="r_row", bufs=2)
                nc.vector.reciprocal(r_row[:], ps_o[D:D + 1, :])
                r_b = n_pool.tile([D, QB], f32, tag="r_b", bufs=2)
                nc.gpsimd.partition_broadcast(r_b[:], r_row[:])
                o_n = n_pool.tile([D, QB], f32, tag="o_n", bufs=2)
                nc.vector.tensor_mul(o_n[:], ps_o[:D, :], r_b[:])
                nc.sync.dma_start(out[b], o_n[:])

    nc.compile()
    return nc


def _prep_inputs(query, key, value, valid):
    import ml_dtypes

    vclamp = np.clip(valid, 1, S)
    k_tiles = tuple(int(x) for x in np.ceil(vclamp / KT).astype(np.int64))

    kxh = np.ascontiguousarray(key.transpose(0, 2, 1)).astype(
        ml_dtypes.bfloat16)  # [B, D, S]
    vxh = np.zeros((B, S, KT), dtype=np.float32)  # padded to 128 weight cols
    vxh[:, :, :D] = value
    vxh[:, :, D] = 1.0
    for b in range(B):
        vxh[b, vclamp[b]:, :] = 0.0  # masked keys contribute nothing
    # [B, S, 128] -> [B, KT, NKMAX, 128]: per-partition contiguous k-tile runs
    vxt = np.ascontiguousarray(
        vxh.reshape(B, NKMAX, KT, KT).transpose(0, 2, 1, 3)
    ).astype(ml_dtypes.bfloat16)
    qt = query.transpose(0, 2, 1)  # [B, D, S]

    in_maps = []
    for c in range(N_CORES):
        qxh = np.ascontiguousarray(
            qt[:, :, c * QB:(c + 1) * QB].transpose(1, 0, 2)
        ).reshape(D, B * QB).astype(ml_dtypes.bfloat16)
        in_maps.append({"qx": qxh, "kx": kxh, "vx": vxt})
    return k_tiles, in_maps


def kernel(query, key, value, valid_len):
    from concourse.bass_utils import run_bass_kernel_spmd

    query = np.ascontiguousarray(query, dtype=np.float32)
    key = np.ascontiguousarray(key, dtype=np.float32)
    value = np.ascontiguousarray(value, dtype=np.float32)
    valid = np.asarray(valid_len).astype(np.int64)
    assert query.shape == (B, S, D) and key.shape == (B, S, D)
    assert value.shape == (B, S, D) and valid.shape == (B,)

    k_tiles, in_maps = _prep_inputs(query, key, value, valid)

    nc = _PROGRAM_CACHE.get(k_tiles)
    if nc is None:
        nc = _build_program(k_tiles)
        _PROGRAM_CACHE[k_tiles] = nc

    res = run_bass_kernel_spmd(nc, in_maps, core_ids=list(range(N_CORES)))

    full = np.empty((B, S, D), dtype=np.float32)
    for c in range(N_CORES):
        # out is [B, D, QB]; transpose back
        full[:, c * QB:(c + 1) * QB, :] = res.results[c]["out"].transpose(0, 2, 1)

    # valid_len == 0 never occurs per the spec (randint >= 1), but the
    # reference would produce uniform attention there; match it exactly.
    if np.any(valid < 1):
        for b in np.nonzero(valid < 1)[0]:
            sc = (query[b] @ key[b].T) * SCALE - 1.0e6
            a = np.exp(sc - sc.max(axis=-1, keepdims=True))
            a /= a.sum(axis=-1, keepdims=True)
            full[b] = a @ value[b]

    return full


# revision 10
# speedup vs baseline: 1.0426x; 1.0426x over previous
"""Masked dot-product attention on 8 Trainium2 NeuronCores.

Problem: B=8, S=4096, D=64 fp32; per-batch key-length mask; softmax over keys.

Sharding: sequence-parallel over Q rows. Each core computes a 512-row Q slice
of all 8 batches. The key loop for batch b runs ceil(valid_len[b]/128) tiles
(same trip counts on every core -> one SPMD program, perfectly balanced
regardless of the valid_len distribution).

Per (batch, core) unit, with scores kept in transposed [k, q] layout:
  phase 1: psum_s[k=128, q=512] = K_tile.T @ Q  (bf16, fast weight load)
  exp:     one ScalarE activation per group of 3 k-tiles, exp(0.125 * s),
           PSUM -> SBUF bf16. No max-subtraction: scores ~ N(0,1) after the
           1/8 scale, exp is safe in fp32.
  phase 2: psum_o[128, q=512] += V_tile.T @ exp_tile in bf16. V is padded to
           128 weight columns (FWL) with col 64 = ones, so row 64 of psum_o
           accumulates the softmax denominator.
  tail:    r = 1/psum_o[64] (DVE), partition-broadcast r (GpSimd), multiply
           rows 0..63 (DVE), DMA out in [d, q] layout; host transposes back.

Masking costs nothing on-device: the host zeroes V rows (incl. the ones
column) at key positions >= valid_len, so masked keys contribute 0 to both
numerator and denominator; exp of their scores is finite garbage times zero.

Perf notes baked in below: per-batch coalesced DMAs (small DMAs are
descriptor-dominated and serialize on the issue queue); a scratch-matmul
warm-up burst so the PE HAM clock gate opens (1.2 -> 2.4 GHz) before real
work; largest batches first so the exposed tail batch is small;
double-buffered 3-bank PSUM score groups keep PE and ACT both dense.
"""

import math
from contextlib import ExitStack

import numpy as np

B = 8
S = 4096
D = 64
N_CORES = 8
QB = S // N_CORES  # 512 q rows per core per batch
KT = 128  # k rows per tile
NKMAX = S // KT  # 32
GROUP = 3  # k-tiles per PSUM group / ACT instruction
SCALE = 1.0 / math.sqrt(D)

_PROGRAM_CACHE: dict = {}


def _build_program(k_tiles):
    import concourse.tile as tile
    from concourse import bacc, mybir

    f32 = mybir.dt.float32
    bf16 = mybir.dt.bfloat16
    nc = bacc.Bacc("TRN2", target_bir_lowering=False, debug=False,
                   enable_asserts=False, num_devices=N_CORES)

    qx = nc.dram_tensor("qx", [D, B * QB], bf16, kind="ExternalInput").ap()
    kx = nc.dram_tensor("kx", [B, D, S], bf16, kind="ExternalInput").ap()
    vx = nc.dram_tensor("vx", [B, KT, NKMAX, KT], bf16,
                        kind="ExternalInput").ap()
    out = nc.dram_tensor("out", [B, D, QB], f32, kind="ExternalOutput").ap()

    with tile.TileContext(nc) as tc:
        with ExitStack() as ctx:
            q_pool = ctx.enter_context(tc.tile_pool(name="q", bufs=1))
            k_pool = ctx.enter_context(tc.tile_pool(name="k", bufs=2))
            v_pool = ctx.enter_context(tc.tile_pool(name="v", bufs=2))
            e_pool = ctx.enter_context(tc.tile_pool(name="e", bufs=3))
            n_pool = ctx.enter_context(tc.tile_pool(name="n", bufs=2))
            ps_s_pool = ctx.enter_context(
                tc.tile_pool(name="ps_s", bufs=2, space="PSUM"))
            ps_o_pool = ctx.enter_context(
                tc.tile_pool(name="ps_o", bufs=2, space="PSUM"))

            q_all = q_pool.tile([D, B * QB], bf16)
            nc.sync.dma_start(q_all[:], qx[:])

            # HAM warm-up: dense scratch matmuls while the first DMAs land,
            # so the PE clock ungates (1.2 -> 2.4 GHz) before real work.
            wu_sb = q_pool.tile([D, QB], bf16, tag="warm", bufs=1)
            nc.gpsimd.memset(wu_sb[:], 0.0)
            ps_w = ps_o_pool.tile([KT, QB], f32, tag="ps_o")
            for _ in range(24):
                nc.tensor.matmul(ps_w[:], lhsT=wu_sb[:, :KT],
                                 rhs=wu_sb[:], start=True, stop=True)

            # largest batches first; the exposed tail batch is the smallest.
            for b in sorted(range(B), key=lambda x: -k_tiles[x]):
                nk = k_tiles[b]
                ngroups = (nk + GROUP - 1) // GROUP

                k_all = k_pool.tile([D, S], bf16)
                nc.sync.dma_start(k_all[:, :nk * KT], kx[b][:, :nk * KT])
                v_all = v_pool.tile([KT, NKMAX * KT], bf16)
                nc.sync.dma_start(
                    v_all[:, :nk * KT].rearrange("p (t c) -> p t c", c=KT),
                    vx[b][:, :nk, :])

                ps_o = ps_o_pool.tile([KT, QB], f32, tag="ps_o")
                q_b = q_all[:, b * QB:(b + 1) * QB]

                for g in range(ngroups):
                    tg = min(GROUP, nk - g * GROUP)
                    ps_s = ps_s_pool.tile([KT, GROUP * QB], f32)
                    e_sb = e_pool.tile([KT, GROUP * QB], bf16)
                    for tl in range(tg):
                        kt = g * GROUP + tl
                        nc.tensor.matmul(
                            ps_s[:, tl * QB:(tl + 1) * QB],
                            lhsT=k_all[:, kt * KT:(kt + 1) * KT],
                            rhs=q_b,
                            start=True, stop=True)
                    nc.scalar.activation(
                        e_sb[:, :tg * QB], ps_s[:, :tg * QB],
                        mybir.ActivationFunctionType.Exp, scale=SCALE)
                    for tl in range(tg):
                        kt = g * GROUP + tl
                        nc.tensor.matmul(
                            ps_o[:],
                            lhsT=v_all[:, kt * KT:(kt + 1) * KT],
                            rhs=e_sb[:, tl * QB:(tl + 1) * QB],
                            start=(kt == 0), stop=(kt == nk - 1),
                            skip_group_check=True)

                r_row = n_pool.tile([1, QB], f32, tag="r_row", bufs=2)
                nc.vector.reciprocal(r_row[:], ps_o[D:D + 1, :])
                r_b = n_pool.tile([D, QB], f32, tag="r_b", bufs=2)
                nc.gpsimd.partition_broadcast(r_b[:], r_row[:])
                o_n = n_pool.tile([D, QB], f32, tag="o_n", bufs=2)
                nc.vector.tensor_mul(o_n[:], ps_o[:D, :], r_b[:])
                nc.sync.dma_start(out[b], o_n[:])

    nc.compile()
    return nc


def _prep_inputs(query, key, value, valid):
    import ml_dtypes

    vclamp = np.clip(valid, 1, S)
    k_tiles = tuple(int(x) for x in np.ceil(vclamp / KT).astype(np.int64))

    kxh = np.ascontiguousarray(key.transpose(0, 2, 1)).astype(
        ml_dtypes.bfloat16)  # [B, D, S]
    vxh = np.zeros((B, S, KT), dtype=np.float32)  # padded to 128 weight cols
    vxh[:, :, :D] = value
    vxh[:, :, D] = 1.0
    for b in range(B):
        vxh[b, vclamp[b]:, :] = 0.0  # masked keys contribute nothing
    # [B, S, 128] -> [B, KT, NKMAX, 128]: per-partition contiguous k-tile runs
    vxt = np.ascontiguousarray(
        vxh.reshape(B, NKMAX, KT, KT).transpose(0, 2, 1, 3)
    ).astype(ml_dtypes.bfloat16)
    qt = query.transpose(0, 2, 1)  # [B, D, S]

    in_maps = []
    for c in range(N_CORES):
        qxh = np.ascontiguousarray(
            qt[:, :, c * QB:(c + 1) * QB].transpose(1, 0, 2)
        ).reshape(D, B * QB).astype(ml_dtypes.bfloat16)
        in_maps.append({"qx": qxh, "kx": kxh, "vx": vxt})
    return k_tiles, in_maps


def kernel(query, key, value, valid_len):
    from concourse.bass_utils import run_bass_kernel_spmd

    query = np.ascontiguousarray(query, dtype=np.float32)
    key = np.ascontiguousarray(key, dtype=np.float32)
    value = np.ascontiguousarray(value, dtype=np.float32)
    valid = np.asarray(valid_len).astype(np.int64)
    assert query.shape == (B, S, D) and key.shape == (B, S, D)
    assert value.shape == (B, S, D) and valid.shape == (B,)

    k_tiles, in_maps = _prep_inputs(query, key, value, valid)

    nc = _PROGRAM_CACHE.get(k_tiles)
    if nc is None:
        nc = _build_program(k_tiles)
        _PROGRAM_CACHE[k_tiles] = nc

    res = run_bass_kernel_spmd(nc, in_maps, core_ids=list(range(N_CORES)))

    full = np.empty((B, S, D), dtype=np.float32)
    for c in range(N_CORES):
        # out is [B, D, QB]; transpose back
        full[:, c * QB:(c + 1) * QB, :] = res.results[c]["out"].transpose(0, 2, 1)

    # valid_len == 0 never occurs per the spec (randint >= 1), but the
    # reference would produce uniform attention there; match it exactly.
    if np.any(valid < 1):
        for b in np.nonzero(valid < 1)[0]:
            sc = (query[b] @ key[b].T) * SCALE - 1.0e6
            a = np.exp(sc - sc.max(axis=-1, keepdims=True))
            a /= a.sum(axis=-1, keepdims=True)
            full[b] = a @ value[b]

    return full
